# revision 31
# baseline (speedup 1.0000x reference)
"""GAT (2-layer, 8-head then 1-head) Bass/Tile kernel for Trainium2, 8 NeuronCores.

Sharding: nodes are sorted by in-degree and dealt round-robin to the 8 cores, so
every core sees a nearly identical degree profile and edge count.  Each core
owns the incoming edges of its nodes, laid out densely as
[dst-node-partition x degree-slot]; softmax denominators and weighted feature
sums are then plain free-dim reductions (no scatter / segment ops on device).
Per-edge source-node features are fetched with dma_gather from a replicated
node table (bf16 rows, packed two nodes per 512B row so the int16 gather index
is the pair id; a parity-predicated copy picks the right half).  The attention
projections (a_src/a_dst) are folded into the dense weight matmuls, so the
gathered row already carries [h | a_src].  An AllGather shares each layer's
node table between cores.

v2: all edge compute stays in the gather-native [partition=dst-node,
slot-column, feature] layout (no feature-major transpose pass).  Features are
packed c-major (head index innermost) so per-head broadcasts keep the DVE in
2x bf16 mode; parity selects run on int32 bitcasts to halve their 1x-mode
element count; leaky-relu runs on the scalar engine (Lrelu); pad slots point
at a sentinel table row with a_src=-300 so they vanish from the softmax
without a mask multiply; slot pairs are added (2x mode) before the always-1x
tensor_reduce.
"""

import os
import sys

import numpy as np

for _p in ("/opt/trn_rl_repo",):
    if _p not in sys.path:
        sys.path.insert(0, _p)

import concourse.bacc as bacc
import concourse.bass as bass
import concourse.mybir as mybir
import concourse.tile as tile
from concourse import bass2jax as _bass2jax
from concourse.bass_utils import run_bass_kernel_spmd

# surface compile-hook exceptions (PJRT swallows the python traceback)
if not getattr(_bass2jax, "_gat_hook_wrapped", False):
    _orig_cc_hook = _bass2jax.neuronx_cc_hook

    def _verbose_cc_hook(*a, **kw):
        try:
            return _orig_cc_hook(*a, **kw)
        except BaseException:
            import traceback

            traceback.print_exc()
            sys.stdout.flush()
            sys.stderr.flush()
            raise

    _bass2jax.neuronx_cc_hook = _verbose_cc_hook
    _bass2jax._gat_hook_wrapped = True
from concourse.masks import make_identity
from contextlib import ExitStack

FP32 = mybir.dt.float32
BF16 = mybir.dt.bfloat16
I16 = mybir.dt.int16
I32 = mybir.dt.int32
U8 = mybir.dt.uint8

N_CORES = 8
P = 128
NEG_SLOPE = 0.2
EPS = 1e-16
SENT = -300.0  # sentinel a_src for pad slots: exp(lrelu(SENT+a_dst)) == 0

# layer-1: IN=128, HID=8, H=8 ; layer-2: 64 -> 40, 1 head
IN_DIM = 128
H1, C1 = 8, 8
HID1 = H1 * C1  # 64
OUT_DIM = 40

# node-table sub-row layouts (bf16), h packed c-major (head innermost):
#   T1 row: [h(64) | a_src(8) | pad(56)]    = 128 bf16 = 256B; pair = 512B elem
#   T2 row: [h2(40) | a_src2 | a_src2 | pad] = 64 bf16 = 128B; pair = 256B elem
T1_ROW = 128
T2_ROW = 64

CAP = int(os.environ.get("GAT_CAP", "64"))  # chunk width in slot-columns


def _degree_layout(dst, n_nodes, n_cores):
    """Host-side layout: degree-sorted round-robin node assignment plus a
    shared per-block slot-count profile (identical for all cores)."""
    nodes_per_core = -(-n_nodes // (n_cores * P)) * P  # ceil to block multiple
    n_pad = nodes_per_core * n_cores
    deg = np.zeros(n_pad, dtype=np.int64)
    np.add.at(deg, dst, 1)
    order = np.argsort(-deg, kind="stable")  # node ids by degree desc
    rank = np.empty(n_pad, dtype=np.int64)
    rank[order] = np.arange(n_pad)
    # table position of node v: core = rank % n_cores, local = rank // n_cores
    core_of = rank % n_cores
    loc_of = rank // n_cores
    tablepos = core_of * nodes_per_core + loc_of
    n_blocks = nodes_per_core // P
    # per-block slot count: max degree among the block's nodes over all cores
    # == max degree among ranks [j*128*n_cores, (j+1)*128*n_cores)
    deg_by_rank = deg[order]
    d_blocks = []
    for j in range(n_blocks):
        d = int(deg_by_rank[j * P * n_cores : (j + 1) * P * n_cores].max())
        d = max(2, d + (d & 1))  # even, >= 2
        d_blocks.append(d)
    return {
        "nodes_per_core": nodes_per_core,
        "n_pad": n_pad,
        "deg": deg,
        "order": order,
        "core_of": core_of,
        "loc_of": loc_of,
        "tablepos": tablepos,
        "n_blocks": n_blocks,
        "d_blocks": d_blocks,
    }


def _edge_streams(src, dst, lay, n_cores):
    """Build per-core gather-index / parity streams.  Pad slots point at the
    sentinel pair (rows n_pad, n_pad+1) whose a_src is SENT."""
    npc = lay["nodes_per_core"]
    d_blocks = lay["d_blocks"]
    n_blocks = lay["n_blocks"]
    c_total = int(np.sum(d_blocks))
    s_total = c_total * P
    col0 = np.concatenate([[0], np.cumsum(d_blocks)])
    sent_pair = lay["n_pad"] >> 1

    core_of, loc_of, tablepos = lay["core_of"], lay["loc_of"], lay["tablepos"]
    # order edges by destination core / local node
    e_core = core_of[dst]
    e_loc = loc_of[dst]
    idx = [np.full(s_total, sent_pair, dtype=np.int16) for _ in range(n_cores)]
    par = [np.zeros(s_total, dtype=np.uint8) for _ in range(n_cores)]
    srcpos = tablepos[src]
    e_sort = np.lexsort((e_loc, e_core))
    e_core_s = e_core[e_sort]
    e_loc_s = e_loc[e_sort]
    e_srcpos_s = srcpos[e_sort]
    core_starts = np.searchsorted(e_core_s, np.arange(n_cores + 1))
    for k in range(n_cores):
        a, b = core_starts[k], core_starts[k + 1]
        locs = e_loc_s[a:b]
        sps = e_srcpos_s[a:b]
        # slot index within each node's run (edges already sorted by loc)
        uniq, first = np.unique(locs, return_index=True)
        slot = np.arange(b - a) - first[np.searchsorted(uniq, locs)]
        blk = locs // P
        n_in_blk = locs % P
        pos = (col0[blk] + slot) * P + n_in_blk
        idx[k][pos] = (sps >> 1).astype(np.int16)
        par[k][pos] = (sps & 1).astype(np.uint8)
    return {
        "c_total": c_total,
        "s_total": s_total,
        "col0": col0,
        "idx": idx,
        "par": par,
    }


def _wrap_idx(flat):
    """int16 stream -> [128, len/16] wrapped (16-partition wrap, replicated x8)."""
    w = flat.reshape(-1, 16).T  # [16, len/16]
    return np.tile(w, (8, 1)).copy()


def _col_major(flat):
    """per-slot stream -> [128, n_cols]; position p = col*128 + partition."""
    return np.ascontiguousarray(flat.reshape(-1, P).T)


def _bf16(x):
    import ml_dtypes

    return x.astype(ml_dtypes.bfloat16)


def _build_program(n_cores, npc, d_blocks, c_total, table_rows):
    """Emit the SPMD Bass/Tile program (identical for every core)."""
    nc = bacc.Bacc(
        "TRN2",
        target_bir_lowering=False,
        debug=False,
        num_devices=n_cores,
        num_swdge_queues=int(os.environ.get("GAT_NSWQ", "4")),
    )
    n_blocks = len(d_blocks)
    s_total = c_total * P
    t_rows = table_rows + 2  # + sentinel pair

    # --- I/O ---
    xT = nc.dram_tensor("xT", [IN_DIM, npc], BF16, kind="ExternalInput")
    xTf = nc.dram_tensor("xTf", [IN_DIM, table_rows], BF16, kind="ExternalInput")
    wcat = nc.dram_tensor("wcat", [IN_DIM, 80], BF16, kind="ExternalInput")
    w2cat = nc.dram_tensor("w2cat", [HID1, 43], BF16, kind="ExternalInput")
    bias1 = nc.dram_tensor("bias1", [P, HID1], FP32, kind="ExternalInput")
    bias2 = nc.dram_tensor("bias2", [P, OUT_DIM], FP32, kind="ExternalInput")
    idx16 = nc.dram_tensor("idx16", [P, s_total // 16], I16, kind="ExternalInput")
    par_d = nc.dram_tensor("par", [P, c_total], U8, kind="ExternalInput")
    out2 = nc.dram_tensor("out2", [npc, OUT_DIM], FP32, kind="ExternalOutput")

    t2_loc = nc.dram_tensor("t2_loc", [npc, T2_ROW], BF16)
    t1_loc = nc.dram_tensor("t1_loc", [npc, T1_ROW], BF16)
    t1_full = nc.dram_tensor("t1_full", [t_rows, T1_ROW], BF16, addr_space="Shared")
    t2_full = nc.dram_tensor("t2_full", [t_rows, T2_ROW], BF16, addr_space="Shared")

    phases = os.environ.get("GAT_PHASES", "ABC")
    depth = int(os.environ.get("GAT_DEPTH", "9"))
    nswq = int(os.environ.get("GAT_NSWQ", "4"))
    single_packet = bool(os.environ.get("GAT_SP"))
    col0 = np.concatenate([[0], np.cumsum(d_blocks)])
    replica = [list(range(n_cores))]

    # chunks of consecutive equal-d blocks, capped at CAP slot-columns
    chunks = []  # (j0, nb, d, c_lo)
    j = 0
    while j < n_blocks:
        d = d_blocks[j]
        j1 = j
        while j1 < n_blocks and d_blocks[j1] == d and (j1 - j + 1) * d <= CAP:
            j1 += 1
        chunks.append((j, j1 - j, d, int(col0[j])))
        j = j1
    if not os.environ.get("GAT_NOREVC"):
        chunks = chunks[::-1]  # smallest-d chunks last: shorter layer tails
    NBM = max(nb for _, nb, _, _ in chunks)

    with tile.TileContext(nc) as tc, ExitStack() as ctx:
        consts = ctx.enter_context(tc.tile_pool(name="consts", bufs=1))
        wcat_s = consts.tile([IN_DIM, 80], BF16)
        nc.sync.dma_start(wcat_s[:], wcat[:])
        w2cat_s = consts.tile([HID1, 43], BF16)
        nc.sync.dma_start(w2cat_s[:], w2cat[:])
        w2dbl_s = consts.tile([P, 43], BF16)
        nc.sync.dma_start(w2dbl_s[0:HID1, :], w2cat[:])
        nc.sync.dma_start(w2dbl_s[HID1:P, :], w2cat[:])
        b1_s = consts.tile([P, HID1], FP32)
        nc.sync.dma_start(b1_s[:], bias1[:])
        b2_s = consts.tile([P, OUT_DIM], FP32)
        nc.sync.dma_start(b2_s[:], bias2[:])
        ident = consts.tile([P, P], BF16)
        make_identity(nc, ident[:])
        # per-block a_dst columns kept on-chip from the producing phase
        adst1_s = consts.tile([P, n_blocks * H1], BF16)
        adst2_s = consts.tile([P, n_blocks], BF16)
        idx_s = consts.tile([P, s_total // 16], I16)
        nc.sync.dma_start(idx_s[:], idx16[:])
        par_s = consts.tile([P, c_total], U8)
        nc.sync.dma_start(par_s[:], par_d[:])
        # sentinel rows: [h=0 | a_src=SENT | 0]
        sent1 = consts.tile([2, T1_ROW], BF16)
        nc.vector.memset(sent1[:], 0.0)
        nc.vector.memset(sent1[:, HID1 : HID1 + H1], SENT)
        sent2 = consts.tile([2, T2_ROW], BF16)
        nc.vector.memset(sent2[:], 0.0)
        nc.vector.memset(sent2[:, OUT_DIM : OUT_DIM + 2], SENT)

        def edge_chunk(gb, half, ch, asw, a_off, adst_b, j0, nb, d, c_lo,
                       sel_pool, sm_pool, dn_all, ft_all):
            """Process one chunk of nb consecutive blocks sharing slot width d,
            entirely in gather-native [P, slot-col, feature] layout.  Writes
            per-block softmax denominators into dn_all and unnormalized
            feature sums into ft_all."""
            ncol = nb * d
            if depth < 1:
                return
            omit = os.environ.get("GAT_OMIT", "")
            par_b = par_s[:, c_lo : c_lo + ncol]
            # --- parity selects (int32 bitcast halves 1x-mode elem count)
            sel_a = sm_pool.tile([P, CAP, asw], BF16, tag="sel_a")
            sel_h = sel_pool.tile([P, CAP, ch], BF16, tag="sel_h")
            if "S" in omit:
                nc.vector.memset(sel_a[:, 0:ncol, :], 0)
                nc.vector.memset(sel_h[:, 0:ncol, :], 0)
            else:
                nc.vector.tensor_copy(
                    sel_a[:, 0:ncol, :].bitcast(I32),
                    gb[:, 0:ncol, a_off : a_off + asw].bitcast(I32),
                )
                nc.vector.copy_predicated(
                    sel_a[:, 0:ncol, :].bitcast(I32),
                    par_b.unsqueeze(2).to_broadcast([P, ncol, asw // 2]),
                    gb[:, 0:ncol, half + a_off : half + a_off + asw].bitcast(I32),
                )
                nc.vector.tensor_copy(
                    sel_h[:, 0:ncol, :].bitcast(I32),
                    gb[:, 0:ncol, 0:ch].bitcast(I32),
                )
                nc.vector.copy_predicated(
                    sel_h[:, 0:ncol, :].bitcast(I32),
                    par_b.unsqueeze(2).to_broadcast([P, ncol, ch // 2]),
                    gb[:, 0:ncol, half : half + ch].bitcast(I32),
                )
            if depth < 2:
                return
            # --- e = lrelu(a_src + a_dst); ex = exp(e)  (ACT engine)
            ex_t = sm_pool.tile([P, CAP, asw], BF16, tag="ex_t")
            if "E" in omit:
                nc.vector.memset(ex_t[:, 0:ncol, :], 1)
            else:
                e_t = sm_pool.tile([P, CAP, asw], BF16, tag="e_t")
                nc.vector.tensor_tensor(
                    out=e_t[:, 0:ncol, :].rearrange("p (b d) h -> p b d h", b=nb),
                    in0=sel_a[:, 0:ncol, :].rearrange("p (b d) h -> p b d h", b=nb),
                    in1=adst_b,
                    op=mybir.AluOpType.add,
                )
                e2_t = sm_pool.tile([P, CAP, asw], BF16, tag="e2_t")
                nc.vector.scalar_tensor_tensor(
                    out=e2_t[:, 0:ncol, :],
                    in0=e_t[:, 0:ncol, :],
                    scalar=NEG_SLOPE,
                    op0=mybir.AluOpType.mult,
                    in1=e_t[:, 0:ncol, :],
                    op1=mybir.AluOpType.max,
                )
                nc.scalar.activation(
                    ex_t[:, 0:ncol, :], e2_t[:, 0:ncol, :],
                    mybir.ActivationFunctionType.Exp,
                )
            # --- denominator (strided view; reduce is 1x regardless)
            if "D" in omit:
                nc.vector.memset(dn_all[:, j0 : j0 + nb, :], 1)
            else:
                nc.vector.reduce_sum(
                    dn_all[:, j0 : j0 + nb, :],
                    ex_t[:, 0:ncol, :]
                    .rearrange("p (b d) h -> p b d h", b=nb)
                    .transpose([0, 1, 3, 2]),
                    axis=mybir.AxisListType.X,
                )
            # --- unnormalized weighted features: wm = h * ex (2x: bcast on c)
            wm = sel_pool.tile([P, CAP, ch], BF16, tag="wm")
            if "W" in omit:
                nc.vector.memset(wm[:, 0:ncol, :], 0)
            else:
                nc.vector.tensor_tensor(
                    out=wm[:, 0:ncol, :].rearrange("p n (c k) -> p n c k", k=asw),
                    in0=sel_h[:, 0:ncol, :].rearrange("p n (c k) -> p n c k", k=asw),
                    in1=ex_t[:, 0:ncol, :]
                    .unsqueeze(2)
                    .to_broadcast([P, ncol, ch // asw, asw]),
                    op=mybir.AluOpType.mult,
                )
            # --- slot-pair add (2x) then 1x reduce on half the data
            if "F" in omit:
                nc.vector.memset(ft_all[:, j0 : j0 + nb, :], 0)
            else:
                pa = sm_pool.tile([P, CAP // 2, ch], BF16, tag="pa")
                wv = wm[:, 0:ncol, :].rearrange("p (x two) f -> p x (two f)", two=2)
                nc.vector.tensor_tensor(
                    out=pa[:, 0 : ncol // 2, :],
                    in0=wv[:, :, 0:ch],
                    in1=wv[:, :, ch : 2 * ch],
                    op=mybir.AluOpType.add,
                )
                nc.vector.reduce_sum(
                    ft_all[:, j0 : j0 + nb, :],
                    pa[:, 0 : ncol // 2, :]
                    .rearrange("p (b e) f -> p b e f", b=nb)
                    .transpose([0, 1, 3, 2]),
                    axis=mybir.AxisListType.X,
                )

        GSUB = int(os.environ.get("GAT_GSUB", "12"))
        GSUB2 = int(os.environ.get("GAT_GSUB2", "8"))

        def sub_gathers(gb, tv, elem, ncol, c_lo, qctr, gsub=None):
            """Fill gb[:, 0:ncol, :] with several independent sub-gathers on
            rotating SWDGE queues (deepens DMA pipelining)."""
            step = gsub or GSUB
            for a in range(0, ncol, step):
                b = min(a + step, ncol)
                nc.gpsimd.dma_gather(
                    out_ap=gb[:, a:b, :],
                    in_ap=tv,
                    idxs_ap=idx_s[:, (c_lo + a) * 8 : (c_lo + b) * 8],
                    num_idxs=(b - a) * P,
                    num_idxs_reg=(b - a) * P,
                    elem_size=elem,
                    elem_step=elem,
                    single_packet=single_packet,
                    queue_num=qctr[0] % nswq,
                )
                qctr[0] += 1

        def emit_phases():
            # ---------- phase A: replicated full T1 table (no collective) ----
            # Every core computes the FULL [h | a_src | a_dst] table from the
            # replicated x (PE matmul is cheap) and writes its local t1_full;
            # cols 80:128 stay garbage (never read).  A small second pass over
            # the core's own x extracts per-block a_dst for the e-chain.
            BGRP = 6  # blocks per PSUM bank (6*80*4B = 1920B < 2KB)
            all_blocks = table_rows // P
            with (
                tc.tile_pool(name="pa_sb", bufs=3) as pa_sb,
                tc.tile_pool(name="pa_ps", bufs=2, space="PSUM") as pa_ps,
            ):
                if "A" in phases and not os.environ.get("GAT_T1REPL"):
                    # default: per-core table + AllGather (fastest on HW; the
                    # replicated-phase-A variant below measured slower despite
                    # a better cost-model estimate)
                    xs0 = pa_sb.tile([IN_DIM, npc], BF16, tag="xs0")
                    nc.sync.dma_start(xs0[:], xT[:])
                    t1sb = pa_sb.tile([P, n_blocks, T1_ROW], BF16, tag="t1sb")
                    for g0 in range(0, n_blocks, BGRP):
                        nb = min(BGRP, n_blocks - g0)
                        ps = pa_ps.tile([P, BGRP * 80], FP32, tag="ps")
                        for b in range(nb):
                            jj = g0 + b
                            nc.tensor.matmul(
                                ps[:, b * 80 : (b + 1) * 80],
                                lhsT=xs0[:, jj * P : (jj + 1) * P],
                                rhs=wcat_s[:],
                                start=True,
                                stop=True,
                            )
                        psv = ps[:, 0 : nb * 80].rearrange("p (b f) -> p b f", b=nb)
                        nc.vector.tensor_copy(t1sb[:, g0 : g0 + nb, 0:80], psv)
                        nc.vector.tensor_copy(
                            adst1_s[:, g0 * H1 : (g0 + nb) * H1].rearrange(
                                "p (b h) -> p b h", h=H1
                            ),
                            psv[:, :, 72:80],
                        )
                    nc.sync.dma_start(
                        t1_loc[:].rearrange("(j p) f -> p j f", p=P), t1sb[:]
                    )
                    nc.gpsimd.collective_compute(
                        "AllGather",
                        mybir.AluOpType.bypass,
                        replica_groups=replica,
                        ins=[t1_loc[:]],
                        outs=[t1_full[0:table_rows, :]],
                    )
                elif "A" in phases:
                    AG2 = 4 * BGRP  # blocks per DMA super-group
                    for s0 in range(0, all_blocks, AG2):
                        nbs = min(AG2, all_blocks - s0)
                        xs = pa_sb.tile([IN_DIM, AG2 * P], BF16, tag="xs")
                        nc.sync.dma_start(
                            xs[:, 0 : nbs * P], xTf[:, s0 * P : (s0 + nbs) * P]
                        )
                        t1g = pa_sb.tile([P, AG2, 80], BF16, tag="t1g")
                        for g0 in range(0, nbs, BGRP):
                            nb = min(BGRP, nbs - g0)
                            ps = pa_ps.tile([P, BGRP * 80], FP32, tag="ps")
                            for b in range(nb):
                                nc.tensor.matmul(
                                    ps[:, b * 80 : (b + 1) * 80],
                                    lhsT=xs[:, (g0 + b) * P : (g0 + b + 1) * P],
                                    rhs=wcat_s[:],
                                    start=True,
                                    stop=True,
                                )
                            nc.vector.tensor_copy(
                                t1g[:, g0 : g0 + nb, :],
                                ps[:, 0 : nb * 80].rearrange("p (b f) -> p b f", b=nb),
                            )
                        nc.sync.dma_start(
                            t1_full[s0 * P : (s0 + nbs) * P, 0:80].rearrange(
                                "(j p) f -> p j f", p=P
                            ),
                            t1g[:, 0:nbs, :],
                        )
                    # own a_dst pass (8-col matmuls on the core's own x)
                    xo = pa_sb.tile([IN_DIM, npc], BF16, tag="xo")
                    nc.sync.dma_start(xo[:], xT[:])
                    for g0 in range(0, n_blocks, BGRP):
                        nb = min(BGRP, n_blocks - g0)
                        psd = pa_ps.tile([P, BGRP * H1], FP32, tag="psd")
                        for b in range(nb):
                            jj = g0 + b
                            nc.tensor.matmul(
                                psd[:, b * H1 : (b + 1) * H1],
                                lhsT=xo[:, jj * P : (jj + 1) * P],
                                rhs=wcat_s[:, 72:80],
                                start=True,
                                stop=True,
                            )
                        nc.vector.tensor_copy(
                            adst1_s[:, g0 * H1 : (g0 + nb) * H1].rearrange(
                                "p (b h) -> p b h", h=H1
                            ),
                            psd[:, 0 : nb * H1].rearrange("p (b f) -> p b f", b=nb),
                        )
            nc.sync.dma_start(t1_full[table_rows : table_rows + 2, :], sent1[:])

            # ---------- phase B: layer-1 edges + build T2 ----------
            t1v = t1_full[:].rearrange("(a b) c -> a (b c)", b=2)  # [pairs, 256]
            gbufs = int(os.environ.get("GAT_GBUFS", "3"))
            with (
                tc.tile_pool(name="pb_gb", bufs=gbufs) as pb_gb,
                tc.tile_pool(name="pb_sel", bufs=2) as pb_sel,
                tc.tile_pool(name="pb_sm", bufs=2) as pb_sm,
                tc.tile_pool(name="pb_out", bufs=1) as pb_out,
                tc.tile_pool(name="pb_ps", bufs=2, space="PSUM") as pb_ps,
                tc.tile_pool(name="pb_ps2", bufs=2, space="PSUM") as pb_ps2,
            ):
                if "B" in phases:
                    dn1 = pb_out.tile([P, n_blocks, H1], FP32, tag="dn1")
                    ft1 = pb_out.tile([P, n_blocks, HID1], FP32, tag="ft1")
                    t2sb = pb_out.tile([P, n_blocks, T2_ROW], BF16, tag="t2sb")
                    qctr = [0]
                    for ci, (j0, nb, d, c_lo) in enumerate(chunks):
                        gb = pb_gb.tile([P, CAP, 256], BF16, tag="gb1")
                        sub_gathers(gb, t1v, 256, nb * d, c_lo, qctr)
                        adst_b = (
                            adst1_s[:, j0 * H1 : (j0 + nb) * H1]
                            .rearrange("p (b h) -> p b h", h=H1)
                            .unsqueeze(2)
                            .to_broadcast([P, nb, d, H1])
                        )
                        edge_chunk(gb, 128, HID1, H1, HID1, adst_b, j0, nb, d,
                                   c_lo, pb_sel, pb_sm, dn1, ft1)
                    if depth >= 3:
                        # layer tail: normalize, bias, ELU, project to T2 rows
                        nc.vector.tensor_scalar_add(dn1[:], dn1[:], EPS)
                        rc = pb_out.tile([P, n_blocks, H1], FP32, tag="rc")
                        nc.vector.reciprocal(rc[:], dn1[:])
                        nc.vector.tensor_tensor(
                            out=ft1[:].rearrange("p b (c k) -> p b c k", k=H1),
                            in0=ft1[:].rearrange("p b (c k) -> p b c k", k=H1),
                            in1=rc[:].unsqueeze(2).to_broadcast([P, n_blocks, C1, H1]),
                            op=mybir.AluOpType.mult,
                        )
                        nc.vector.tensor_tensor(
                            out=ft1[:],
                            in0=ft1[:],
                            in1=b1_s[:].unsqueeze(1).to_broadcast([P, n_blocks, HID1]),
                            op=mybir.AluOpType.add,
                        )
                        rl = pb_out.tile([P, n_blocks, HID1], BF16, tag="rl")
                        nc.vector.tensor_scalar_max(rl[:], ft1[:], 0.0)
                        mn = pb_out.tile([P, n_blocks, HID1], BF16, tag="mn")
                        nc.vector.tensor_scalar_min(mn[:], ft1[:], 0.0)
                        nc.scalar.activation(
                            mn[:], mn[:], mybir.ActivationFunctionType.Exp
                        )
                        h1c = pb_out.tile([P, n_blocks, HID1], BF16, tag="h1c")
                        nc.vector.scalar_tensor_tensor(
                            out=h1c[:],
                            in0=mn[:],
                            scalar=-1.0,
                            op0=mybir.AluOpType.add,
                            in1=rl[:],
                            op1=mybir.AluOpType.add,
                        )
                        BG = 6
                        use_ptr = bool(os.environ.get("GAT_PTR"))
                        for g0 in range(0, n_blocks, BG):
                            nbg = min(BG, n_blocks - g0)
                            ps2 = pb_ps2.tile([P, BG * 43], FP32, tag="ps2")
                            b = 0
                            while b < nbg:
                                if use_ptr and b + 1 < nbg:
                                    # two 64-feature blocks per 128x128 transpose
                                    tp2 = pb_ps.tile([P, P], BF16, tag="tp2")
                                    nc.tensor.transpose(
                                        out=tp2[:],
                                        in_=h1c[:, g0 + b : g0 + b + 2, :].rearrange(
                                            "p b f -> p (b f)"
                                        ),
                                        identity=ident[:],
                                    )
                                    h1T2 = pb_sm.tile([P, P], BF16, tag="h1T2")
                                    nc.vector.tensor_copy(h1T2[:], tp2[:])
                                    nc.tensor.matmul(
                                        ps2[:, b * 43 : (b + 1) * 43],
                                        lhsT=h1T2[0:HID1, :],
                                        rhs=w2cat_s[:],
                                        start=True,
                                        stop=True,
                                    )
                                    nc.tensor.matmul(
                                        ps2[:, (b + 1) * 43 : (b + 2) * 43],
                                        lhsT=h1T2[HID1:P, :],
                                        rhs=w2dbl_s[HID1:P, :],
                                        start=True,
                                        stop=True,
                                    )
                                    b += 2
                                    continue
                                tp = pb_ps.tile([HID1, P], BF16, tag="tp")
                                nc.tensor.transpose(
                                    out=tp[:], in_=h1c[:, g0 + b, :], identity=ident[:]
                                )
                                h1T = pb_sm.tile([HID1, P], BF16, tag="h1T")
                                nc.vector.tensor_copy(h1T[:], tp[:])
                                nc.tensor.matmul(
                                    ps2[:, b * 43 : (b + 1) * 43],
                                    lhsT=h1T[:],
                                    rhs=w2cat_s[:],
                                    start=True,
                                    stop=True,
                                )
                                b += 1
                            ps2v = ps2[:, 0 : nbg * 43].rearrange(
                                "p (b f) -> p b f", b=nbg
                            )
                            nc.vector.tensor_copy(
                                t2sb[:, g0 : g0 + nbg, 0:42], ps2v[:, :, 0:42]
                            )
                            nc.vector.tensor_copy(
                                adst2_s[:, g0 : g0 + nbg].unsqueeze(2),
                                ps2v[:, :, 42:43],
                            )
                        nc.vector.memset(t2sb[:, :, 42:T2_ROW], 0)
                        nc.sync.dma_start(
                            t2_loc[:].rearrange("(j p) f -> p j f", p=P), t2sb[:]
                        )
            if not os.environ.get("GAT_NOCC"):
                nc.gpsimd.collective_compute(
                    "AllGather",
                    mybir.AluOpType.bypass,
                    replica_groups=replica,
                    ins=[t2_loc[:]],
                    outs=[t2_full[0:table_rows, :]],
                )
            nc.sync.dma_start(t2_full[table_rows : table_rows + 2, :], sent2[:])

            # ---------- phase C: layer-2 edges ----------
            t2v = t2_full[:].rearrange("(a b) c -> a (b c)", b=2)  # [pairs, 128]
            gbufs2 = int(os.environ.get("GAT_GBUFS2", "4"))
            with (
                tc.tile_pool(name="pc_gb", bufs=gbufs2) as pc_gb,
                tc.tile_pool(name="pc_sel", bufs=2) as pc_sel,
                tc.tile_pool(name="pc_sm", bufs=2) as pc_sm,
                tc.tile_pool(name="pc_out", bufs=1) as pc_out,
            ):
                if "C" in phases:
                    dn2 = pc_out.tile([P, n_blocks, 2], FP32, tag="dn2")
                    ft2 = pc_out.tile([P, n_blocks, OUT_DIM], FP32, tag="ft2")
                    qctr = [0]
                    for ci, (j0, nb, d, c_lo) in enumerate(chunks):
                        gb2 = pc_gb.tile([P, CAP, 128], BF16, tag="gb2")
                        sub_gathers(gb2, t2v, 128, nb * d, c_lo, qctr, gsub=GSUB2)
                        adst_b = (
                            adst2_s[:, j0 : j0 + nb]
                            .unsqueeze(2)
                            .unsqueeze(3)
                            .to_broadcast([P, nb, d, 2])
                        )
                        edge_chunk(gb2, 64, OUT_DIM, 2, OUT_DIM, adst_b, j0, nb,
                                   d, c_lo, pc_sel, pc_sm, dn2, ft2)
                    if depth >= 4:
                        nc.vector.tensor_scalar_add(dn2[:], dn2[:], EPS)
                        rc2 = pc_out.tile([P, n_blocks, 2], FP32, tag="rc2")
                        nc.vector.reciprocal(rc2[:], dn2[:])
                        nc.vector.tensor_tensor(
                            out=ft2[:].rearrange("p b (c k) -> p b c k", k=2),
                            in0=ft2[:].rearrange("p b (c k) -> p b c k", k=2),
                            in1=rc2[:].unsqueeze(2).to_broadcast(
                                [P, n_blocks, OUT_DIM // 2, 2]
                            ),
                            op=mybir.AluOpType.mult,
                        )
                        o2sb = pc_out.tile([P, n_blocks, OUT_DIM], FP32, tag="o2sb")
                        nc.vector.tensor_tensor(
                            out=o2sb[:],
                            in0=ft2[:],
                            in1=b2_s[:].unsqueeze(1).to_broadcast(
                                [P, n_blocks, OUT_DIM]
                            ),
                            op=mybir.AluOpType.add,
                        )
                        nc.sync.dma_start(
                            out2[:].rearrange("(j p) f -> p j f", p=P), o2sb[:]
                        )

        for _rep in range(int(os.environ.get("GAT_REPEAT", "1"))):
            emit_phases()

    nc.compile()
    return nc


_CACHE = {}
LAST_RESULTS = None
LAST_EXEC_S = None


def kernel(**inputs) -> np.ndarray:
    x = np.asarray(inputs["x"], dtype=np.float32)
    edge_index = np.asarray(inputs["edge_index"])
    W1 = np.asarray(inputs["W1"], dtype=np.float32)
    att_src1 = np.asarray(inputs["att_src1"], dtype=np.float32)
    att_dst1 = np.asarray(inputs["att_dst1"], dtype=np.float32)
    b1 = np.asarray(inputs["bias1"], dtype=np.float32)
    W2 = np.asarray(inputs["W2"], dtype=np.float32)
    att_src2 = np.asarray(inputs["att_src2"], dtype=np.float32)
    att_dst2 = np.asarray(inputs["att_dst2"], dtype=np.float32)
    b2 = np.asarray(inputs["bias2"], dtype=np.float32)

    n_nodes = x.shape[0]
    src = np.asarray(edge_index[0], dtype=np.int64)
    dst = np.asarray(edge_index[1], dtype=np.int64)

    lay = _degree_layout(dst, n_nodes, N_CORES)
    streams = _edge_streams(src, dst, lay, N_CORES)
    npc = lay["nodes_per_core"]
    table_rows = lay["n_pad"]

    key = (npc, tuple(lay["d_blocks"]), streams["c_total"], table_rows)
    if key not in _CACHE:
        _CACHE[key] = _build_program(
            N_CORES, npc, lay["d_blocks"], streams["c_total"], table_rows
        )
    nc = _CACHE[key]

    # host-side parameter folding; h features packed c-major (head innermost)
    perm = np.array([(f % H1) * C1 + f // H1 for f in range(HID1)])
    # new column j = c*H1 + h  <->  old column h*C1 + c
    W1p = W1[:, perm]
    a_src1 = np.stack(
        [W1[:, h * C1 : (h + 1) * C1] @ att_src1[h] for h in range(H1)], axis=1
    )  # [128, 8]
    a_dst1 = np.stack(
        [W1[:, h * C1 : (h + 1) * C1] @ att_dst1[h] for h in range(H1)], axis=1
    )
    wcat = _bf16(np.concatenate([W1p, a_src1, a_dst1], axis=1))  # [128, 80]
    W2p = W2[perm, :]
    as2 = W2p @ att_src2[0][:, None]
    ad2 = W2p @ att_dst2[0][:, None]
    w2cat = _bf16(np.concatenate([W2p, as2, as2, ad2], axis=1))  # [64, 43]
    b1p = b1[perm]

    # per-core inputs
    in_maps = []
    x_pad = np.zeros((lay["n_pad"], IN_DIM), dtype=np.float32)
    x_pad[:n_nodes] = x
    # full x in table order (row r of table = node order[(r % npc)*8 + r // npc])
    r = np.arange(lay["n_pad"])
    rank_r = (r % npc) * N_CORES + r // npc
    xTf = np.ascontiguousarray(_bf16(x_pad[lay["order"][rank_r]].T))
    for k in range(N_CORES):
        ranks = np.arange(k, lay["n_pad"], N_CORES)
        node_ids = lay["order"][ranks]
        xk = x_pad[node_ids]  # [npc, 128]
        in_maps.append(
            {
                "xT": np.ascontiguousarray(_bf16(xk.T)),
                "xTf": xTf,
                "wcat": wcat,
                "w2cat": w2cat,
                "bias1": np.tile(b1p.reshape(1, -1), (P, 1)),
                "bias2": np.tile(b2.reshape(1, -1), (P, 1)),
                "idx16": _wrap_idx(streams["idx"][k]),
                "par": _col_major(streams["par"][k]),
            }
        )

    if os.environ.get("GAT_BASS_SIM"):
        from concourse.bass_interp import MultiCoreSim

        sim = MultiCoreSim(nc, num_cores=N_CORES, trace=False)
        for k in range(N_CORES):
            for name, arr in in_maps[k].items():
                sim.cores[k].tensor(name)[:] = arr
        sim.simulate(check_with_hw=False)
        results = [{"out2": np.array(sim.cores[k].tensor("out2"))} for k in range(N_CORES)]
    else:
        import time as _time

        _t0 = _time.time()
        res = run_bass_kernel_spmd(
            nc,
            in_maps,
            list(range(N_CORES)),
            trace=bool(os.environ.get("GAT_BASS_TRACE")),
        )
        global LAST_RESULTS, LAST_EXEC_S
        LAST_EXEC_S = _time.time() - _t0
        results = res.results
        LAST_RESULTS = res

    out = np.zeros((n_nodes, OUT_DIM), dtype=np.float32)
    for k in range(N_CORES):
        ranks = np.arange(k, lay["n_pad"], N_CORES)
        node_ids = lay["order"][ranks]
        ok = results[k]["out2"]
        keep = node_ids < n_nodes
        out[node_ids[keep]] = ok[keep]
    return out
